# revision 19
# baseline (speedup 1.0000x reference)
"""Trainium2 Bass kernel for a fake-quantized MLP (qlinear -> gelu -> qlinear).

Reference semantics (B,S,C,H = 32,1024,1024,4096):
    x2d = x.reshape(-1, C)
    h   = round(x2d/sx) @ round(w1/sw1).T * (sx*sw1) + b1 ;  s = max(amax,eps)/127
    g   = gelu(h, exact erf)
    y   = round(g/sh) @ round(w2/sw2).T * (sh*sw2) + b2

Strategy: data-parallel over rows across 8 cores.  Quantized ints fit
exactly in bf16, so matmuls run at full bf16 rate with exact fp32 PSUM
accumulation.

v2 layout (vs v1 baseline):
  - weight amax scans are SHARDED (each core scans 1/8 of w1 and w2 via
    extra per-core inputs w1s/w2s); the three maxes (x, w1, w2) ride in a
    single AllReduce(max) of a [3,1] buffer, so sw2 is known up front.
  - all quant transposes for x and w1 run on the DMA XBAR
    (dma_start_transpose, 16x128 tiles) instead of PE identity matmuls,
    so phase A's PE stream is pure matmul+ldweights.
  - the x amax scan runs in reverse row order and retains the last 4
    tiles (rows 0..1023 = chunk 0) in SBUF, so chunk 0 skips its reload.
  - h (gelu output) is staged to DRAM transposed as (H, rows) in FP16
    (halves the 128MB h roundtrip; adds ~0.2% noise, well within 2e-2).
  - the hmax AllReduce is emitted BEFORE the w2 quant+transpose so it
    flies while the idle-PE window does the w2 transposes (in the v1
    ordering the AR's PE preduce sat behind 256 transpose matmuls).
  - h chunk 0 of phase B is prefetched (f16 loads don't depend on sh).
"""

import sys

import numpy as np

try:
    import concourse.bass as bass
except ImportError:  # pragma: no cover
    sys.path.insert(0, "/opt/trn_rl_repo")
    import concourse.bass as bass

import concourse.mybir as mybir
from contextlib import ExitStack
import concourse.tile as tile
from concourse import masks
from concourse.bass_utils import run_bass_kernel_spmd

from concourse.bass import _add_dep_helper as _add_dep

F32 = mybir.dt.float32
BF16 = mybir.dt.bfloat16
F16 = mybir.dt.float16
AF = mybir.ActivationFunctionType
ALU = mybir.AluOpType

QP = 127.0
EPS = 1e-6
MAGIC = 12582912.0  # 1.5 * 2**23: fp32 round-to-nearest-even integer trick

# full problem shapes
B, S, C, H = 32, 1024, 1024, 4096
N_CORES = 8


def _split_matmul_waits(nc):
    """This toolchain's walrus codegen allows only ONE sync-wait slot per
    lowered instruction (Matmult waits all land on its LDWEIGHTS since
    --enable-ldw-opt=false; queue DMAs use a single-slot DIRECT2D struct).
    Peel extra waits onto same-engine NoOps inserted just before, except for
    framework-generated drain/barrier instructions which support many."""
    n_split = 0
    for f in nc.m.functions:
        for bb in f.blocks:
            insts = bb.instructions
            out = []
            changed = False
            for inst in insts:
                si = getattr(inst, "sync_info", None)
                if si is not None and si.on_wait and len(si.on_wait) > 1:
                    waits = list(si.on_wait)
                    for k, w in enumerate(waits[:-1]):
                        nop = mybir.InstNoOp(
                            name=f"{inst.name}-wsplit{k}", ins=[], outs=[]
                        )
                        nop.engine = inst.engine
                        nop.sync_info = mybir.SyncInfo(
                            on_wait=[w], on_update=[]
                        )
                        out.append(nop)
                    inst.sync_info = mybir.SyncInfo(
                        on_wait=[waits[-1]], on_update=list(si.on_update or [])
                    )
                    n_split += 1
                    changed = True
                out.append(inst)
            if changed:
                bb.instructions = out
    return n_split


def _dedup_ldweights(nc):
    """Tile legalization emits explicit Ldweights+Matmult pairs, and walrus
    runs with --enable-ldw-opt=false, so every matmul re-streams its
    stationary operand (128 extra PE cycles on a 512-cycle matmul).  Drop an
    Ldweights whose weights AP is identical to the previous one on the PE
    stream (the PE array still holds that stationary); keep its semaphore
    effects on a NoOp."""
    n = 0
    for f in nc.m.functions:
        for bb in f.blocks:
            insts = bb.instructions
            out = []
            last_key = None
            changed = False
            for inst in insts:
                if isinstance(inst, mybir.InstLdweights):
                    key = str(inst.ins[0])
                    if key == last_key:
                        si = getattr(inst, "sync_info", None)
                        if si is not None and (si.on_wait or si.on_update):
                            nop = mybir.InstNoOp(
                                name=inst.name + "-lw", ins=[], outs=[]
                            )
                            nop.engine = inst.engine
                            nop.sync_info = si
                            out.append(nop)
                        n += 1
                        changed = True
                        continue
                    last_key = key
                elif isinstance(inst, mybir.InstMatmult):
                    if inst.is_transpose or getattr(inst, "ldweights", None):
                        last_key = None
                out.append(inst)
            if changed:
                bb.instructions = out
    return n


def build_nc(rows=4096, c=C, h=H, n_cores=N_CORES, gelu="Gelu", split_waits=True):
    """Build the per-core SPMD Bass program.

    rows: rows of x2d handled by each core.
    gelu: "Gelu" (HW ACT table), "Erf" (x*(0.5*erf(x/sqrt2)+0.5)),
          "Identity" (for simulator runs; CoreSim lacks Gelu/Erf).
    """
    assert rows % 1024 == 0 and c == 1024 and h % 512 == 0
    nc = bass.Bass()

    x_in = nc.dram_tensor("x", [rows, c], F32, kind="ExternalInput")
    w1_in = nc.dram_tensor("w1", [h, c], F32, kind="ExternalInput")
    b1_in = nc.dram_tensor("b1", [h], F32, kind="ExternalInput")
    w2_in = nc.dram_tensor("w2", [c, h], F32, kind="ExternalInput")
    b2_in = nc.dram_tensor("b2", [c], F32, kind="ExternalInput")
    w1s_in = nc.dram_tensor("w1s", [h // n_cores, c], F32, kind="ExternalInput")
    w2s_in = nc.dram_tensor("w2s", [c // n_cores, h], F32, kind="ExternalInput")
    y_out = nc.dram_tensor("y", [rows, c], F32, kind="ExternalOutput")

    ct = c // 128   # c in 128-blocks (8)
    ht = h // 128   # h in 128-blocks (32)
    CH = 1024       # phase A m-chunk
    n_ch = rows // CH
    NMS = CH // 512  # 512-wide matmul groups per chunk (2)
    n_chunk = rows // 512  # phase B m-chunks
    groups = [list(range(n_cores))]

    with tile.TileContext(nc) as tc, ExitStack() as top:
        consts = top.enter_context(tc.tile_pool(name="consts", bufs=1))
        scal = top.enter_context(tc.tile_pool(name="scal", bufs=1))
        dram = top.enter_context(tc.tile_pool(name="dram", bufs=1, space="DRAM"))

        ident = consts.tile([128, 128], BF16)
        masks.make_identity(nc, ident[:])
        ident_f = consts.tile([128, 128], F32)
        masks.make_identity(nc, ident_f[:])

        # b1 as (128, ht): b1_sb[p, jb] = b1[jb*128 + p]
        b1_sb = consts.tile([128, ht], F32)
        nc.sync.dma_start(
            out=b1_sb[:], in_=b1_in.ap().rearrange("(a b) -> b a", b=128)
        )

        magic_b = consts.tile([128, 1], F32)
        nc.vector.memset(magic_b[:], MAGIC)

        # h scratch in DRAM, transposed: (h, rows), FP16
        h_dram = dram.tile([h, rows], F16)
        # collective bounce buffers (DRAM, non-IO)
        ar3_in = dram.tile([3, 1], F32, tag="ar3i")
        ar3_out = dram.tile([3, 1], F32, tag="ar3o")
        arh_in = dram.tile([1, 1], F32, tag="arhi")
        arh_out = dram.tile([1, 1], F32, tag="arho")

        # ---------- phase 0: local amaxes ----------
        # acc3 columns: 0 = x, 1 = w1 (shard), 2 = w2 (shard).  Weight scans
        # are sharded across cores; all three maxes share one AllReduce.
        acc3 = scal.tile([128, 3], F32)
        nc.vector.memset(acc3[:], 0.0)

        # xhold retains the last 4 x-scan tiles (rows 0..1023 = phase A
        # chunk 0); it lives until phase A ends, so it opens before the
        # scan pools (LIFO pool discipline).
        xhold_stack = ExitStack()
        xhold = xhold_stack.enter_context(tc.tile_pool(name="xhold", bufs=4))
        xhold_tiles = {}

        sc_stack = ExitStack()
        s_p = sc_stack.enter_context(tc.tile_pool(name="s_p", bufs=3))
        s_r = sc_stack.enter_context(tc.tile_pool(name="s_r", bufs=4))
        # w1 shard scan: (h/8, c) = (512, 1024) -> one [128, 4, 1024] tile
        t = s_p.tile([128, 4 * c], F32, tag="w1s")
        nc.sync.dma_start(
            out=t[:].rearrange("b (a c) -> b a c", a=4),
            in_=w1s_in.ap().rearrange("(a b) c -> b a c", b=128),
        )
        r = s_r.tile([128, 1], F32, tag="w1sr")
        nc.vector.tensor_reduce(
            out=r[:], in_=t[:], axis=mybir.AxisListType.X, op=ALU.max,
            apply_absolute_value=True,
        )
        nc.vector.tensor_tensor(
            out=acc3[:, 1:2], in0=acc3[:, 1:2], in1=r[:], op=ALU.max
        )
        # w2 shard scan: (c/8, h) = (128, 4096) -> one [128, 4096] tile
        t = s_p.tile([128, h], F32, tag="w2s")
        nc.sync.dma_start(out=t[:], in_=w2s_in[:, :])
        r = s_r.tile([128, 1], F32, tag="w2sr")
        nc.vector.tensor_reduce(
            out=r[:], in_=t[:], axis=mybir.AxisListType.X, op=ALU.max,
            apply_absolute_value=True,
        )
        nc.vector.tensor_tensor(
            out=acc3[:, 2:3], in0=acc3[:, 2:3], in1=r[:], op=ALU.max
        )

        # x scan in REVERSE order; the last 4 tiles (rows 0..1023 = phase A
        # chunk 0) are retained in xhold so chunk 0 skips its reload.
        # Tiles are [128, 2, 1024] (256 rows) matching the quant pipeline.
        n_xt = rows // 256  # 16
        with tc.tile_pool(name="xscan", bufs=4) as xsp:
            for k in range(n_xt - 1, -1, -1):
                pool = xhold if k < 4 else xsp
                t = pool.tile([128, 2, c], F32, tag="xsc", name=f"xsc{k}")
                nc.sync.dma_start(
                    out=t[:],
                    in_=x_in[k * 256 : (k + 1) * 256, :].rearrange(
                        "(a b) c -> b a c", b=128
                    ),
                )
                if k < 4:
                    xhold_tiles[k] = t
                r = s_r.tile([128, 1], F32, tag="xscr")
                nc.vector.tensor_reduce(
                    out=r[:], in_=t[:].rearrange("b a c -> b (a c)"),
                    axis=mybir.AxisListType.X, op=ALU.max,
                    apply_absolute_value=True,
                )
                nc.vector.tensor_tensor(
                    out=acc3[:, 0:1], in0=acc3[:, 0:1], in1=r[:], op=ALU.max
                )

        # cross-partition reduce of all three maxes via one PE matmul
        with tc.tile_pool(name="psR", bufs=1, space="PSUM") as psR:
            pt = psR.tile([3, 128], F32, tag="psR3")
            nc.tensor.matmul(
                pt[:], lhsT=acc3[:], rhs=ident_f[:], start=True, stop=True
            )
            m3 = scal.tile([3, 1], F32, name="m3")
            nc.vector.tensor_reduce(
                out=m3[:], in_=pt[:], axis=mybir.AxisListType.X, op=ALU.max
            )
        nc.gpsimd.dma_start(out=ar3_in[:], in_=m3[:])
        nc.gpsimd.collective_compute(
            "AllReduce", ALU.max, replica_groups=groups,
            ins=[ar3_in.opt()], outs=[ar3_out.opt()],
        )
        sc_stack.close()

        def _derive(bcast_src_dram, name):
            b = scal.tile([128, 1], F32, name=name + "_b")
            nc.sync.dma_start(out=b[:], in_=bcast_src_dram.to_broadcast((128, 1)))
            s = scal.tile([128, 1], F32, name="s_" + name)
            nc.vector.tensor_scalar(
                out=s[:], in0=b[:], scalar1=EPS, scalar2=float(1.0 / QP),
                op0=ALU.max, op1=ALU.mult,
            )
            inv = scal.tile([128, 1], F32, name="inv_" + name)
            nc.vector.reciprocal(out=inv[:], in_=s[:])
            return s, inv

        sx, inv_sx = _derive(ar3_out[0:1, :], "x")
        sw1, inv_sw1 = _derive(ar3_out[1:2, :], "w1")
        sw2, inv_sw2 = _derive(ar3_out[2:3, :], "w2")
        sxw1 = scal.tile([128, 1], F32)
        nc.vector.tensor_tensor(out=sxw1[:], in0=sx[:], in1=sw1[:], op=ALU.mult)

        hmax = scal.tile([128, 1], F32)
        nc.vector.memset(hmax[:], 0.0)

        # ---------- pools for phase A ----------
        pha_stack = ExitStack()
        # w1qT[rb]: [128(c within cb), 8 cb, 128(h within rb)];
        # lhsT slice for (cb, jb=rb) is w1qT[rb][:, cb, :].
        w1qT_pool = pha_stack.enter_context(
            tc.tile_pool(name="w1qT", bufs=ht, side="right")
        )
        w1qT = [
            w1qT_pool.tile([128, ct, 128], BF16, tag="w1qT", name=f"w1qT{i}")
            for i in range(ht)
        ]
        # xqT per (chunk, ms): [128(c within cb), 4 t8, 8 cb, 128(m)];
        # rhs slice for (cb, ms) is xqT[...][:, :, cb, :] (free = 512).
        xqT_pool = pha_stack.enter_context(tc.tile_pool(name="xqT", bufs=2 * NMS))
        xf = pha_stack.enter_context(tc.tile_pool(name="xf", bufs=2))
        xbq = pha_stack.enter_context(tc.tile_pool(name="xbq", bufs=2))
        wf = pha_stack.enter_context(tc.tile_pool(name="wf", bufs=3))
        wq = pha_stack.enter_context(tc.tile_pool(name="wq", bufs=4))
        gs = pha_stack.enter_context(tc.tile_pool(name="gs", bufs=8))
        gr = pha_stack.enter_context(tc.tile_pool(name="gr", bufs=8))
        psH = pha_stack.enter_context(tc.tile_pool(name="psH", bufs=6, space="PSUM"))

        def x_quant_tiles(mc):
            return [
                xqT_pool.tile([128, 4, ct, 128], BF16, tag="xqT",
                              name=f"xqT{mc}_{ms}")
                for ms in range(NMS)
            ]

        def emit_x_quant_s1(mc, td):
            """Stage 1 for x double-block td of chunk mc: load (chunk 0
            reuses the retained scan tiles), quant pass1 (ACT), pass2 (DVE).
            Returns the bf16 tile."""
            k = mc * (CH // 256) + td
            if mc == 0:
                t = xhold_tiles[k]
            else:
                t = xf.tile([128, 2, c], F32, tag="xf")
                nc.sync.dma_start(
                    out=t[:],
                    in_=x_in[k * 256 : (k + 1) * 256, :].rearrange(
                        "(a b) c -> b a c", b=128
                    ),
                )
            flat = t[:].rearrange("b a c -> b (a c)")
            nc.scalar.activation(
                out=flat, in_=flat, func=AF.Identity, bias=magic_b[:],
                scale=inv_sx[:],
            )
            q = xbq.tile([128, 2, c], BF16, tag="xbq")
            nc.vector.tensor_scalar_add(
                out=q[:].rearrange("b a c -> b (a c)"), in0=flat,
                scalar1=-MAGIC,
            )
            return q

        def emit_x_quant_s2(tiles, td, q):
            """Stage 2: XBAR transposes of the bf16 double-block (on the
            scalar HWDGE queue so loads on sync aren't blocked)."""
            for a in range(2):
                t8g = td * 2 + a  # 128-row block index within chunk
                ms, wslot = t8g // 4, t8g % 4
                nc.scalar.dma_start_transpose(
                    out=tiles[ms][:, wslot, :, :], in_=q[:, a, :]
                )

        def emit_w1_quant_s1(rb2):
            """Stage 1 for w1 rows [rb2*256, (rb2+1)*256): load + quant."""
            t = wf.tile([128, 2, c], F32, tag="wf")
            nc.sync.dma_start(
                out=t[:],
                in_=w1_in[rb2 * 256 : (rb2 + 1) * 256, :].rearrange(
                    "(a b) c -> b a c", b=128
                ),
            )
            flat = t[:].rearrange("b a c -> b (a c)")
            nc.scalar.activation(
                out=flat, in_=flat, func=AF.Identity, bias=magic_b[:],
                scale=inv_sw1[:],
            )
            q = wq.tile([128, 2, c], BF16, tag="wq")
            nc.vector.tensor_scalar_add(
                out=q[:].rearrange("b a c -> b (a c)"), in0=flat, scalar1=-MAGIC
            )
            return q

        def emit_w1_quant_s2(rb2, q):
            for a in range(2):
                nc.scalar.dma_start_transpose(
                    out=w1qT[rb2 * 2 + a][:], in_=q[:, a, :]
                )

        # ---------- phase A ----------
        # Emission is software-pipelined so each engine's program order
        # matches data arrival: w1 quant blocks and the next chunk's x
        # pipeline are interleaved into the jb matmul loop (a block's
        # XBAR transposes are emitted one slot after its quant pass so
        # the ACT stream never stalls waiting on DVE).
        xq_tiles = x_quant_tiles(0)
        xq0 = [emit_x_quant_s1(0, td) for td in range(CH // 256)]
        for td, q in enumerate(xq0):
            emit_x_quant_s2(xq_tiles, td, q)
        w1_pend = {}
        for rb2 in range(3):
            w1_pend[rb2] = emit_w1_quant_s1(rb2)
        emit_w1_quant_s2(0, w1_pend.pop(0))

        for mc in range(n_ch):
            nxt = None
            nxt_pend = {}
            for jb in range(ht):
                if mc == 0:
                    # w1 block pipeline: stage-1 block k at jb=2(k-3),
                    # stage-2 (XBARs) one jb later; ~3 blocks of lookahead
                    # ahead of the jb=2k matmuls that consume w1qT[2k].
                    if jb % 2 == 0:
                        k = jb // 2 + 3
                        if k < ht // 2:
                            w1_pend[k] = emit_w1_quant_s1(k)
                    else:
                        k = (jb - 1) // 2 + 1
                        if k in w1_pend:
                            emit_w1_quant_s2(k, w1_pend.pop(k))
                # trickle next chunk's x quant through this chunk's jb loop
                if mc + 1 < n_ch:
                    if jb in (8, 12, 16, 20):
                        td = (jb - 8) // 4
                        if jb == 8:
                            nxt = x_quant_tiles(mc + 1)
                        nxt_pend[td] = emit_x_quant_s1(mc + 1, td)
                    elif jb in (10, 14, 18, 22):
                        td = (jb - 10) // 4
                        emit_x_quant_s2(nxt, td, nxt_pend.pop(td))
                phs = [
                    psH.tile([128, 512], F32, tag="psH", name=f"psH{mc}_{jb}_{i}")
                    for i in range(NMS)
                ]
                prev = None
                for cb in range(ct):
                    for ms in range(NMS):
                        mmi = nc.tensor.matmul(
                            phs[ms][:],
                            lhsT=w1qT[jb][:, cb, :],
                            rhs=xq_tiles[ms][:, :, cb, :],
                            start=(cb == 0),
                            stop=(cb == ct - 1),
                        )
                        if prev is not None:
                            _add_dep(mmi.ins, prev.ins, sync=False,
                                     reason="ldw-order")
                        prev = mmi
                for ms in range(NMS):
                    ph = phs[ms]
                    g = gs.tile([128, 512], F16, tag="gs")
                    if gelu == "Erf":
                        hh = gs.tile([128, 512], F32, tag="gh")
                        nc.scalar.activation(
                            out=hh[:], in_=ph[:], func=AF.Identity,
                            bias=b1_sb[:, jb : jb + 1], scale=sxw1[:],
                        )
                        e = gs.tile([128, 512], F32, tag="ge")
                        nc.scalar.activation(
                            out=e[:], in_=hh[:], func=AF.Erf, bias=0.0,
                            scale=float(1.0 / np.sqrt(2.0)),
                        )
                        nc.vector.tensor_scalar(
                            out=e[:], in0=e[:], scalar1=0.5, scalar2=0.5,
                            op0=ALU.mult, op1=ALU.add,
                        )
                        nc.vector.tensor_tensor(
                            out=g[:], in0=e[:], in1=hh[:], op=ALU.mult
                        )
                    else:
                        nc.scalar.activation(
                            out=g[:], in_=ph[:], func=getattr(AF, gelu),
                            bias=b1_sb[:, jb : jb + 1], scale=sxw1[:],
                        )
                    r = gr.tile([128, 1], F32, tag="gr")
                    nc.vector.tensor_reduce(
                        out=r[:], in_=g[:], axis=mybir.AxisListType.X,
                        op=ALU.max, apply_absolute_value=True,
                    )
                    nc.vector.tensor_tensor(
                        out=hmax[:], in0=hmax[:], in1=r[:], op=ALU.max
                    )
                    m0 = mc * CH + ms * 512
                    nc.sync.dma_start(
                        out=h_dram[jb * 128 : (jb + 1) * 128, m0 : m0 + 512],
                        in_=g[:],
                    )
            if nxt is not None:
                xq_tiles = nxt

        pha_stack.close()
        xhold_stack.close()

        # ---------- h scale AllReduce (emitted FIRST so it overlaps the
        # w2 quant+transpose below) ----------
        with tc.tile_pool(name="psRh", bufs=1, space="PSUM") as psR:
            hmax_t = psR.tile([1, 128], F32, tag="psRh")
            nc.tensor.matmul(
                hmax_t[:], lhsT=hmax[:], rhs=ident_f[:], start=True, stop=True
            )
            hmax_r = scal.tile([1, 1], F32)
            nc.vector.tensor_reduce(
                out=hmax_r[:], in_=hmax_t[:], axis=mybir.AxisListType.X,
                op=ALU.max,
            )
        nc.gpsimd.dma_start(out=arh_in[:], in_=hmax_r[:])
        nc.gpsimd.collective_compute(
            "AllReduce", ALU.max, replica_groups=groups,
            ins=[arh_in.opt()], outs=[arh_out.opt()],
        )

        # ---------- phase B pools ----------
        phb_stack = ExitStack()
        w2qT_pool = phb_stack.enter_context(tc.tile_pool(name="w2qT", bufs=ht))
        w2qT = [
            w2qT_pool.tile([128, c], BF16, tag="w2qT", name=f"w2qT{i}")
            for i in range(ht)
        ]
        hf = phb_stack.enter_context(tc.tile_pool(name="hf", bufs=8))
        tfp = phb_stack.enter_context(tc.tile_pool(name="tfp", bufs=2))
        b2p = phb_stack.enter_context(tc.tile_pool(name="b2p", bufs=1))
        hqtp = phb_stack.enter_context(tc.tile_pool(name="hqt", bufs=2))
        ys = phb_stack.enter_context(tc.tile_pool(name="ys", bufs=4))

        # prefetch h chunk 0 (f16 loads don't depend on sh)
        def emit_h_load(mc):
            tiles = []
            for j4 in range(ht // 4):
                th = hf.tile([128, 4, 512], F16, tag="hf")
                nc.sync.dma_start(
                    out=th[:],
                    in_=h_dram[
                        j4 * 512 : (j4 + 1) * 512, mc * 512 : (mc + 1) * 512
                    ].rearrange("(a b) m -> b a m", b=128),
                )
                tiles.append(th)
            return tiles

        h_tiles0 = emit_h_load(0)

        b2_b = b2p.tile([128, c], F32)
        nc.sync.dma_start(
            out=b2_b[:],
            in_=b2_in.ap().rearrange("(o a) -> o a", o=1).to_broadcast((128, c)),
        )

        # ---------- w2 quant + transpose (PE identity matmuls; fills the
        # AllReduce window where the PE is otherwise idle) ----------
        with tc.tile_pool(name="w2f", bufs=3) as fp, tc.tile_pool(
            name="w2q", bufs=3
        ) as qp, tc.tile_pool(name="w2ps", bufs=4, space="PSUM") as pp:
            for rb in range(ct):
                for jc in range(h // 1024):
                    t = fp.tile([128, 1024], F32, tag="w2f")
                    nc.sync.dma_start(
                        out=t[:],
                        in_=w2_in[rb * 128 : (rb + 1) * 128,
                                  jc * 1024 : (jc + 1) * 1024],
                    )
                    nc.scalar.activation(
                        out=t[:], in_=t[:], func=AF.Identity, bias=magic_b[:],
                        scale=inv_sw2[:],
                    )
                    q = qp.tile([128, 1024], BF16, tag="w2qq")
                    nc.vector.tensor_scalar_add(out=q[:], in0=t[:], scalar1=-MAGIC)
                    for cb in range(8):
                        ps = pp.tile([128, 128], F32, tag="w2ps")
                        nc.tensor.matmul(
                            ps[:], lhsT=q[:, cb * 128 : (cb + 1) * 128],
                            rhs=ident[:], start=True, stop=True,
                        )
                        nc.vector.tensor_copy(
                            out=w2qT[jc * 8 + cb][:, rb * 128 : (rb + 1) * 128],
                            in_=ps[:],
                        )

        psY = phb_stack.enter_context(
            tc.tile_pool(name="psY", bufs=3 * (c // 512), space="PSUM")
        )

        gh_b = scal.tile([128, 1], F32)
        nc.sync.dma_start(out=gh_b[:], in_=arh_out.to_broadcast((128, 1)))
        sh = scal.tile([128, 1], F32)
        nc.vector.tensor_scalar(
            out=sh[:], in0=gh_b[:], scalar1=EPS, scalar2=float(1.0 / QP),
            op0=ALU.max, op1=ALU.mult,
        )
        inv_sh = scal.tile([128, 1], F32)
        nc.vector.reciprocal(out=inv_sh[:], in_=sh[:])
        shw2 = scal.tile([128, 1], F32)
        nc.vector.tensor_tensor(out=shw2[:], in0=sh[:], in1=sw2[:], op=ALU.mult)

        # ---------- phase B: y = hq.T.T @ w2q.T * (sh*sw2) + b2 ----------
        for mc in range(n_chunk):
            h_tiles = h_tiles0 if mc == 0 else emit_h_load(mc)
            hqT = hqtp.tile([128, ht * 512], BF16, tag="hqT")
            for j4 in range(ht // 4):
                th = h_tiles[j4]
                flat = th[:].rearrange("b a m -> b (a m)")
                tf = tfp.tile([128, 4 * 512], F32, tag="hff")
                nc.scalar.activation(
                    out=tf[:], in_=flat, func=AF.Identity, bias=magic_b[:],
                    scale=inv_sh[:],
                )
                nc.vector.tensor_scalar_add(
                    out=hqT[:, j4 * 2048 : (j4 + 1) * 2048], in0=tf[:],
                    scalar1=-MAGIC,
                )
            n_ob = c // 512
            for ms in range(4):
                pys = [
                    psY.tile([128, 512], F32, tag="psY", name=f"psY{mc}_{ms}_{i}")
                    for i in range(n_ob)
                ]
                prev = None
                for jb in range(ht):
                    for ob in range(n_ob):
                        mmi = nc.tensor.matmul(
                            pys[ob][:],
                            lhsT=hqT[:, jb * 512 + ms * 128 :
                                     jb * 512 + (ms + 1) * 128],
                            rhs=w2qT[jb][:, ob * 512 : (ob + 1) * 512],
                            start=(jb == 0),
                            stop=(jb == ht - 1),
                        )
                        if prev is not None:
                            _add_dep(mmi.ins, prev.ins, sync=False,
                                     reason="ldw-order")
                        prev = mmi
                for ob in range(n_ob):
                    yt = ys.tile([128, 512], F32, tag="ys")
                    nc.vector.scalar_tensor_tensor(
                        out=yt[:], in0=pys[ob][:], scalar=shw2[:],
                        in1=b2_b[:, ob * 512 : (ob + 1) * 512],
                        op0=ALU.mult, op1=ALU.add,
                    )
                    m0 = mc * 512 + ms * 128
                    nc.scalar.dma_start(
                        out=y_out[m0 : m0 + 128, ob * 512 : (ob + 1) * 512],
                        in_=yt[:],
                    )

        phb_stack.close()

    if split_waits:
        _split_matmul_waits(nc)
        _dedup_ldweights(nc)
    return nc


_CACHED = {}


def _get_nc(rows, c, h, n_cores, gelu):
    key = (rows, c, h, n_cores, gelu)
    if key not in _CACHED:
        _CACHED[key] = build_nc(rows=rows, c=c, h=h, n_cores=n_cores, gelu=gelu)
    return _CACHED[key]


def run(inputs, trace=False, gelu="Gelu", n_cores=N_CORES):
    x = np.asarray(inputs["x"], np.float32)
    w1 = np.ascontiguousarray(np.asarray(inputs["w1"], np.float32))
    b1 = np.ascontiguousarray(np.asarray(inputs["b1"], np.float32))
    w2 = np.ascontiguousarray(np.asarray(inputs["w2"], np.float32))
    b2 = np.ascontiguousarray(np.asarray(inputs["b2"], np.float32))
    b_, s_, c_ = x.shape
    h_ = w1.shape[0]
    x2d = np.ascontiguousarray(x.reshape(-1, c_))
    rows = x2d.shape[0] // n_cores
    hs = h_ // n_cores
    cs = c_ // n_cores
    nc = _get_nc(rows, c_, h_, n_cores, gelu)
    in_maps = [
        {
            "x": np.ascontiguousarray(x2d[i * rows : (i + 1) * rows]),
            "w1": w1,
            "b1": b1,
            "w2": w2,
            "b2": b2,
            "w1s": np.ascontiguousarray(w1[i * hs : (i + 1) * hs]),
            "w2s": np.ascontiguousarray(w2[i * cs : (i + 1) * cs]),
        }
        for i in range(n_cores)
    ]
    res = run_bass_kernel_spmd(nc, in_maps, list(range(n_cores)), trace=trace)
    y2d = np.concatenate([r["y"] for r in res.results], axis=0)
    return y2d.reshape(b_, s_, c_).astype(np.float32), res


def kernel(x, w1, b1, w2, b2):
    y, _ = run({"x": x, "w1": w1, "b1": b1, "w2": w2, "b2": b2})
    return y
